# revision 43
# baseline (speedup 1.0000x reference)
"""Trainium2 Bass kernel for nn_Concatenation_90701119357422.

Computes, for full inputs:
    ret  = mean(ret_feat, axis=1) @ Wp.T + bp          # [B, H]
    out  = concat([h, ret[batch]], -1) @ Wl.T + bl     # [N, H]

Strategy (8 cores, data-parallel over N):
  - out = h @ Wl[:, :H].T + ret2[batch]  where  ret2 = ret @ Wl[:, H:].T + bl
  - host casts h to fp16 and pre-transposes it into two feature-major halves
    per core; device runs fp16 matmuls with fp32 PSUM accumulation
  - ret2 is computed on host (tiny) and replicated as a stacked fp8 hi+lo
    table [ret2hi; ret2lo] (error-compensated, near-fp16 precision)
  - per-row gather ret2[batch] is a single fp8 one-hot matmul accumulated
    into the same PSUM tile; the one-hot is built on HOST (rows 0-63 and
    64-127 both hold the one-hot, matching the stacked table) and streamed
    as fp8 -- no on-device broadcast/is_equal work
  - output is written fp16 in a feature-contiguous [128, tiles, H] layout;
    host de-transposes and upcasts to f32
"""

import os
import sys

import numpy as np

for _p in ("/opt/trn_rl_repo", "/root/.axon_site/_ro/trn_rl_repo"):
    if os.path.isdir(_p) and _p not in sys.path:
        sys.path.append(_p)

import concourse.bass as bass
import concourse.mybir as mybir
import concourse.tile as tile
from concourse import bacc
from concourse.bass_utils import run_bass_kernel_spmd

N_TOTAL = 262144
B = 64
K = 16
H = 256
R = 512
N_CORES = 8
SHARD = N_TOTAL // N_CORES  # 32768

CHUNK = 2048                 # rows per pipeline chunk
F32 = mybir.dt.float32
F16 = mybir.dt.float16
F8 = mybir.dt.float8e4
F8NP = mybir.dt.np(F8)


def build_program(shard_rows: int = SHARD):
    assert shard_rows % CHUNK == 0
    n_chunks = shard_rows // CHUNK
    tiles_per_chunk = CHUNK // 128
    n_tiles_total = shard_rows // 128

    nc = bacc.Bacc("TRN2", target_bir_lowering=False, debug=False)

    # feature-major fp16 h halves: hta[k, r] = h[r, k], htb[k, r] = h[r, 128+k]
    hta_d = nc.dram_tensor("hta", [128, shard_rows], F16, kind="ExternalInput").ap()
    htb_d = nc.dram_tensor("htb", [128, shard_rows], F16, kind="ExternalInput").ap()
    ohd = nc.dram_tensor("ohd", [128, shard_rows], F8, kind="ExternalInput").ap()
    wt16 = nc.dram_tensor("wt16", [H, H], F16, kind="ExternalInput").ap()
    r2_d = nc.dram_tensor("r2", [128, H], F8, kind="ExternalInput").ap()
    # out_t[p, t, n] = out[128*t + p, n], fp16; host de-transposes
    out_t = nc.dram_tensor(
        "out_t", [128, n_tiles_total, H], F16, kind="ExternalOutput"
    ).ap()

    with tile.TileContext(nc) as tc:
        with (
            tc.tile_pool(name="const", bufs=1) as cpool,
            tc.tile_pool(name="psum", bufs=1, space="PSUM") as ppool,
            tc.tile_pool(name="ht", bufs=6) as hpool,
            tc.tile_pool(name="oh", bufs=4) as ohpool,
            tc.tile_pool(name="outp", bufs=4) as opool,
        ):
            # ---- constants into SBUF ----
            wt_sb = cpool.tile([128, 2, H], F16)
            nc.scalar.dma_start(wt_sb[:], wt16.rearrange("(kc p) c -> p kc c", p=128))
            ret2_sb = cpool.tile([128, H], F8)
            nc.scalar.dma_start(ret2_sb[:], r2_d[:])

            # ---- main loop ----
            for ci in range(n_chunks):
                r0 = ci * CHUNK
                t0 = ci * tiles_per_chunk
                hta = hpool.tile([128, CHUNK], F16, tag="hta")
                nc.sync.dma_start(out=hta[:], in_=hta_d[:, r0 : r0 + CHUNK])
                htb = hpool.tile([128, CHUNK], F16, tag="htb")
                nc.sync.dma_start(out=htb[:], in_=htb_d[:, r0 : r0 + CHUNK])
                oh = ohpool.tile([128, CHUNK], F8, tag="oh")
                nc.sync.dma_start(out=oh[:], in_=ohd[:, r0 : r0 + CHUNK])

                outsb = opool.tile([128, tiles_per_chunk, H], F16, tag="outsb")
                q_t = tiles_per_chunk // 4
                for j in range(tiles_per_chunk // 2):
                    # two tiles per PSUM bank -> one (wider) copy per pair
                    ps = ppool.tile([128, 2, H], F32, tag="acc", bufs=4)
                    for u in (0, 1):
                        t = 2 * j + u
                        sl = slice(128 * t, 128 * (t + 1))
                        nc.tensor.matmul(
                            ps[:, u], hta[:, sl], wt_sb[:, 0], start=True, stop=False
                        )
                        nc.tensor.matmul(
                            ps[:, u], htb[:, sl], wt_sb[:, 1], start=False, stop=False
                        )
                        nc.tensor.matmul(
                            ps[:, u], oh[:, sl], ret2_sb[:], start=False, stop=True
                        )
                    nc.any.tensor_copy(outsb[:, 2 * j : 2 * j + 2], ps[:])
                    if j % 2 == 1:
                        q = j // 2
                        nc.scalar.dma_start(
                            out=out_t[:, t0 + q * q_t : t0 + (q + 1) * q_t, :],
                            in_=outsb[:, q * q_t : (q + 1) * q_t],
                        )

    nc.compile()
    return nc


def prep_inputs(h, ret_feat, batch, Wp, bp, Wl, bl, shard_rows: int = SHARD,
                n_cores: int = N_CORES):
    """Host-side prep: shard + cast + pre-transpose h. Returns per-core maps."""
    h = np.asarray(h, dtype=np.float32)
    Wl = np.asarray(Wl, dtype=np.float32)
    Wp = np.asarray(Wp, dtype=np.float32)
    bp = np.asarray(bp, dtype=np.float32)
    bl = np.asarray(bl, dtype=np.float32)
    ret_feat = np.asarray(ret_feat, dtype=np.float32)

    h16 = h.astype(np.float16)
    bt_all = np.asarray(batch).astype(np.int64)

    wt16 = np.ascontiguousarray(Wl[:, :H].T).astype(np.float16)
    # replicated pooled ret table: ret2 = (mean_k rf) @ Wp.T + bp) @ Wl[:,H:].T + bl
    wlr_t = Wl[:, H:].astype(np.float64).T  # [R, H]
    ret = ret_feat.astype(np.float64).mean(axis=1) @ Wp.astype(np.float64).T + bp
    ret2 = ret @ wlr_t + bl  # [B, H] float64
    # stacked fp8 hi+lo table: rows 0-63 = fp8(ret2), 64-127 = fp8(residual)
    r2 = np.zeros((128, H), dtype=F8NP)
    r2[:B] = ret2.astype(F8NP)
    r2[B : 2 * B] = (ret2 - r2[:B].astype(np.float64)).astype(F8NP)

    # duplicated one-hot (rows b and 64+b set), fp8, streamed per core
    n_total = shard_rows * n_cores
    oh_all = np.zeros((128, n_total), dtype=F8NP)
    cols = np.arange(n_total)
    oh_all[bt_all, cols] = 1.0
    oh_all[bt_all + B, cols] = 1.0

    in_maps = []
    for i in range(n_cores):
        s = slice(i * shard_rows, (i + 1) * shard_rows)
        hs = h16[s]
        in_maps.append(
            {
                "hta": np.ascontiguousarray(hs[:, :128].T),
                "htb": np.ascontiguousarray(hs[:, 128:].T),
                "ohd": np.ascontiguousarray(oh_all[:, s]),
                "wt16": wt16,
                "r2": r2,
            }
        )
    return in_maps


_PROGRAM_CACHE = {}


def _get_program(shard_rows: int = SHARD):
    if shard_rows not in _PROGRAM_CACHE:
        _PROGRAM_CACHE[shard_rows] = build_program(shard_rows)
    return _PROGRAM_CACHE[shard_rows]


def kernel(h, ret_feat, batch, Wp, bp, Wl, bl):
    nc = _get_program(SHARD)
    in_maps = prep_inputs(h, ret_feat, batch, Wp, bp, Wl, bl)
    res = run_bass_kernel_spmd(nc, in_maps, list(range(N_CORES)))
    outs = []
    for i in range(N_CORES):
        ot = res.results[i]["out_t"]  # [128, n_tiles, H] fp16
        outs.append(ot.transpose(1, 0, 2).reshape(SHARD, H))
    return np.concatenate(outs, axis=0).astype(np.float32)


# revision 44
# speedup vs baseline: 1.0374x; 1.0374x over previous
"""Trainium2 Bass kernel for nn_Concatenation_90701119357422.

Computes, for full inputs:
    ret  = mean(ret_feat, axis=1) @ Wp.T + bp          # [B, H]
    out  = concat([h, ret[batch]], -1) @ Wl.T + bl     # [N, H]

Strategy (8 cores, data-parallel over N):
  - out = h @ Wl[:, :H].T + ret2[batch]  where  ret2 = ret @ Wl[:, H:].T + bl
  - host casts h to fp16 and pre-transposes it into two feature-major halves
    per core; device runs fp16 matmuls with fp32 PSUM accumulation
  - ret2 is computed on host (tiny) and replicated as a stacked fp8 hi+lo
    table [ret2hi; ret2lo] (error-compensated, near-fp16 precision)
  - per-row gather ret2[batch] is a single fp8 one-hot matmul accumulated
    into the same PSUM tile; the one-hot is built on HOST (rows 0-63 and
    64-127 both hold the one-hot, matching the stacked table) and streamed
    as fp8 -- no on-device broadcast/is_equal work
  - output is written fp16 in a feature-contiguous [128, tiles, H] layout;
    host de-transposes and upcasts to f32
"""

import os
import sys

import numpy as np

for _p in ("/opt/trn_rl_repo", "/root/.axon_site/_ro/trn_rl_repo"):
    if os.path.isdir(_p) and _p not in sys.path:
        sys.path.append(_p)

import concourse.bass as bass
import concourse.mybir as mybir
import concourse.tile as tile
from concourse import bacc
from concourse.bass_utils import run_bass_kernel_spmd

N_TOTAL = 262144
B = 64
K = 16
H = 256
R = 512
N_CORES = 8
SHARD = N_TOTAL // N_CORES  # 32768

CHUNK = 2048                 # rows per pipeline chunk
F32 = mybir.dt.float32
F16 = mybir.dt.float16
F8 = mybir.dt.float8e4
F8NP = mybir.dt.np(F8)


def build_program(shard_rows: int = SHARD):
    assert shard_rows % CHUNK == 0
    n_chunks = shard_rows // CHUNK
    tiles_per_chunk = CHUNK // 128
    n_tiles_total = shard_rows // 128

    nc = bacc.Bacc("TRN2", target_bir_lowering=False, debug=False)

    # feature-major fp16 h halves: hta[k, r] = h[r, k], htb[k, r] = h[r, 128+k]
    hta_d = nc.dram_tensor("hta", [128, shard_rows], F16, kind="ExternalInput").ap()
    htb_d = nc.dram_tensor("htb", [128, shard_rows], F16, kind="ExternalInput").ap()
    ohd = nc.dram_tensor("ohd", [128, shard_rows], F8, kind="ExternalInput").ap()
    wt16 = nc.dram_tensor("wt16", [H, H], F16, kind="ExternalInput").ap()
    r2_d = nc.dram_tensor("r2", [128, H], F8, kind="ExternalInput").ap()
    # out_t[p, t, n] = out[128*t + p, n], fp16; host de-transposes
    out_t = nc.dram_tensor(
        "out_t", [128, n_tiles_total, H], F16, kind="ExternalOutput"
    ).ap()

    with tile.TileContext(nc) as tc:
        with (
            tc.tile_pool(name="const", bufs=1) as cpool,
            tc.tile_pool(name="psum", bufs=1, space="PSUM") as ppool,
            tc.tile_pool(name="ht", bufs=6) as hpool,
            tc.tile_pool(name="oh", bufs=4) as ohpool,
            tc.tile_pool(name="outp", bufs=4) as opool,
        ):
            # ---- constants into SBUF ----
            wt_sb = cpool.tile([128, 2, H], F16)
            nc.scalar.dma_start(wt_sb[:], wt16.rearrange("(kc p) c -> p kc c", p=128))
            ret2_sb = cpool.tile([128, H], F8)
            nc.scalar.dma_start(ret2_sb[:], r2_d[:])

            # ---- main loop ----
            for ci in range(n_chunks):
                r0 = ci * CHUNK
                t0 = ci * tiles_per_chunk
                hta = hpool.tile([128, CHUNK], F16, tag="hta")
                nc.sync.dma_start(out=hta[:], in_=hta_d[:, r0 : r0 + CHUNK])
                htb = hpool.tile([128, CHUNK], F16, tag="htb")
                nc.sync.dma_start(out=htb[:], in_=htb_d[:, r0 : r0 + CHUNK])
                oh = ohpool.tile([128, CHUNK], F8, tag="oh")
                nc.sync.dma_start(out=oh[:], in_=ohd[:, r0 : r0 + CHUNK])

                outsb = opool.tile([128, tiles_per_chunk, H], F16, tag="outsb")
                half_t = tiles_per_chunk // 2
                for t in range(tiles_per_chunk):
                    ps = ppool.tile([128, H], F32, tag="acc", bufs=8)
                    sl = slice(128 * t, 128 * (t + 1))
                    nc.tensor.matmul(
                        ps[:], hta[:, sl], wt_sb[:, 0], start=True, stop=False
                    )
                    nc.tensor.matmul(
                        ps[:], htb[:, sl], wt_sb[:, 1], start=False, stop=False
                    )
                    nc.tensor.matmul(
                        ps[:], oh[:, sl], ret2_sb[:], start=False, stop=True
                    )
                    nc.any.tensor_copy(outsb[:, t], ps[:])
                    if t == half_t - 1:
                        nc.scalar.dma_start(
                            out=out_t[:, t0 : t0 + half_t, :],
                            in_=outsb[:, 0:half_t],
                        )

                nc.scalar.dma_start(
                    out=out_t[:, t0 + half_t : t0 + tiles_per_chunk, :],
                    in_=outsb[:, half_t:tiles_per_chunk],
                )

    nc.compile()
    return nc


def prep_inputs(h, ret_feat, batch, Wp, bp, Wl, bl, shard_rows: int = SHARD,
                n_cores: int = N_CORES):
    """Host-side prep: shard + cast + pre-transpose h. Returns per-core maps."""
    h = np.asarray(h, dtype=np.float32)
    Wl = np.asarray(Wl, dtype=np.float32)
    Wp = np.asarray(Wp, dtype=np.float32)
    bp = np.asarray(bp, dtype=np.float32)
    bl = np.asarray(bl, dtype=np.float32)
    ret_feat = np.asarray(ret_feat, dtype=np.float32)

    h16 = h.astype(np.float16)
    bt_all = np.asarray(batch).astype(np.int64)

    wt16 = np.ascontiguousarray(Wl[:, :H].T).astype(np.float16)
    # replicated pooled ret table: ret2 = (mean_k rf) @ Wp.T + bp) @ Wl[:,H:].T + bl
    wlr_t = Wl[:, H:].astype(np.float64).T  # [R, H]
    ret = ret_feat.astype(np.float64).mean(axis=1) @ Wp.astype(np.float64).T + bp
    ret2 = ret @ wlr_t + bl  # [B, H] float64
    # stacked fp8 hi+lo table: rows 0-63 = fp8(ret2), 64-127 = fp8(residual)
    r2 = np.zeros((128, H), dtype=F8NP)
    r2[:B] = ret2.astype(F8NP)
    r2[B : 2 * B] = (ret2 - r2[:B].astype(np.float64)).astype(F8NP)

    # duplicated one-hot (rows b and 64+b set), fp8, streamed per core
    n_total = shard_rows * n_cores
    oh_all = np.zeros((128, n_total), dtype=F8NP)
    cols = np.arange(n_total)
    oh_all[bt_all, cols] = 1.0
    oh_all[bt_all + B, cols] = 1.0

    in_maps = []
    for i in range(n_cores):
        s = slice(i * shard_rows, (i + 1) * shard_rows)
        hs = h16[s]
        in_maps.append(
            {
                "hta": np.ascontiguousarray(hs[:, :128].T),
                "htb": np.ascontiguousarray(hs[:, 128:].T),
                "ohd": np.ascontiguousarray(oh_all[:, s]),
                "wt16": wt16,
                "r2": r2,
            }
        )
    return in_maps


_PROGRAM_CACHE = {}


def _get_program(shard_rows: int = SHARD):
    if shard_rows not in _PROGRAM_CACHE:
        _PROGRAM_CACHE[shard_rows] = build_program(shard_rows)
    return _PROGRAM_CACHE[shard_rows]


def kernel(h, ret_feat, batch, Wp, bp, Wl, bl):
    nc = _get_program(SHARD)
    in_maps = prep_inputs(h, ret_feat, batch, Wp, bp, Wl, bl)
    res = run_bass_kernel_spmd(nc, in_maps, list(range(N_CORES)))
    outs = []
    for i in range(N_CORES):
        ot = res.results[i]["out_t"]  # [128, n_tiles, H] fp16
        outs.append(ot.transpose(1, 0, 2).reshape(SHARD, H))
    return np.concatenate(outs, axis=0).astype(np.float32)


# revision 45
# speedup vs baseline: 1.0761x; 1.0373x over previous
"""Trainium2 Bass kernel for nn_Concatenation_90701119357422.

Computes, for full inputs:
    ret  = mean(ret_feat, axis=1) @ Wp.T + bp          # [B, H]
    out  = concat([h, ret[batch]], -1) @ Wl.T + bl     # [N, H]

Strategy (8 cores, data-parallel over N):
  - out = h @ Wl[:, :H].T + ret2[batch]  where  ret2 = ret @ Wl[:, H:].T + bl
  - host casts h to fp16 and pre-transposes it into two feature-major halves
    per core; device runs fp16 matmuls with fp32 PSUM accumulation
  - ret2 is computed on host (tiny) and replicated as a stacked fp8 hi+lo
    table [ret2hi; ret2lo] (error-compensated, near-fp16 precision)
  - per-row gather ret2[batch] is a single fp8 one-hot matmul accumulated
    into the same PSUM tile; the one-hot is built on HOST (rows 0-63 and
    64-127 both hold the one-hot, matching the stacked table) and streamed
    as fp8 -- no on-device broadcast/is_equal work
  - output is written fp16 in a feature-contiguous [128, tiles, H] layout;
    host de-transposes and upcasts to f32
"""

import os
import sys

import numpy as np

for _p in ("/opt/trn_rl_repo", "/root/.axon_site/_ro/trn_rl_repo"):
    if os.path.isdir(_p) and _p not in sys.path:
        sys.path.append(_p)

import concourse.bass as bass
import concourse.mybir as mybir
import concourse.tile as tile
from concourse import bacc
from concourse.bass_utils import run_bass_kernel_spmd

N_TOTAL = 262144
B = 64
K = 16
H = 256
R = 512
N_CORES = 8
SHARD = N_TOTAL // N_CORES  # 32768

CHUNK = 2048                 # rows per pipeline chunk
F32 = mybir.dt.float32
F16 = mybir.dt.float16
F8 = mybir.dt.float8e4
F8NP = mybir.dt.np(F8)


def build_program(shard_rows: int = SHARD):
    assert shard_rows % CHUNK == 0
    n_chunks = shard_rows // CHUNK
    tiles_per_chunk = CHUNK // 128
    n_tiles_total = shard_rows // 128

    nc = bacc.Bacc("TRN2", target_bir_lowering=False, debug=False)

    # feature-major fp16 h halves: hta[k, r] = h[r, k], htb[k, r] = h[r, 128+k]
    hta_d = nc.dram_tensor("hta", [128, shard_rows], F16, kind="ExternalInput").ap()
    htb_d = nc.dram_tensor("htb", [128, shard_rows], F16, kind="ExternalInput").ap()
    ohd = nc.dram_tensor("ohd", [128, shard_rows], F8, kind="ExternalInput").ap()
    wt16 = nc.dram_tensor("wt16", [H, H], F16, kind="ExternalInput").ap()
    r2_d = nc.dram_tensor("r2", [128, H], F8, kind="ExternalInput").ap()
    # out_t[p, t, n] = out[128*t + p, n], fp16; host de-transposes
    out_t = nc.dram_tensor(
        "out_t", [128, n_tiles_total, H], F16, kind="ExternalOutput"
    ).ap()

    with tile.TileContext(nc) as tc:
        with (
            tc.tile_pool(name="const", bufs=1) as cpool,
            tc.tile_pool(name="psum", bufs=1, space="PSUM") as ppool,
            tc.tile_pool(name="ht", bufs=6) as hpool,
            tc.tile_pool(name="oh", bufs=4) as ohpool,
            tc.tile_pool(name="outp", bufs=4) as opool,
        ):
            # ---- constants into SBUF ----
            wt_sb = cpool.tile([128, 2, H], F16)
            nc.scalar.dma_start(wt_sb[:], wt16.rearrange("(kc p) c -> p kc c", p=128))
            ret2_sb = cpool.tile([128, H], F8)
            nc.scalar.dma_start(ret2_sb[:], r2_d[:])

            # ---- main loop ----
            for ci in range(n_chunks):
                r0 = ci * CHUNK
                t0 = ci * tiles_per_chunk
                hta = hpool.tile([128, CHUNK], F16, tag="hta")
                nc.sync.dma_start(out=hta[:], in_=hta_d[:, r0 : r0 + CHUNK])
                htb = hpool.tile([128, CHUNK], F16, tag="htb")
                nc.sync.dma_start(out=htb[:], in_=htb_d[:, r0 : r0 + CHUNK])
                oh = ohpool.tile([128, CHUNK], F8, tag="oh")
                nc.sync.dma_start(out=oh[:], in_=ohd[:, r0 : r0 + CHUNK])

                outsb = opool.tile([128, tiles_per_chunk, H], F16, tag="outsb")
                half_t = tiles_per_chunk // 2
                for t in range(tiles_per_chunk):
                    ps = ppool.tile([128, H], F32, tag="acc", bufs=8)
                    sl = slice(128 * t, 128 * (t + 1))
                    nc.tensor.matmul(
                        ps[:], hta[:, sl], wt_sb[:, 0], start=True, stop=False
                    )
                    nc.tensor.matmul(
                        ps[:], htb[:, sl], wt_sb[:, 1], start=False, stop=False
                    )
                    nc.tensor.matmul(
                        ps[:], oh[:, sl], ret2_sb[:], start=False, stop=True
                    )
                    nc.any.tensor_copy(outsb[:, t], ps[:])
                    if t == half_t - 1:
                        # output triggers live on the (otherwise idle) gpsimd
                        # queue so their copy-waits never stall scalar's copies
                        nc.gpsimd.dma_start(
                            out=out_t[:, t0 : t0 + half_t, :],
                            in_=outsb[:, 0:half_t],
                        )

                nc.gpsimd.dma_start(
                    out=out_t[:, t0 + half_t : t0 + tiles_per_chunk, :],
                    in_=outsb[:, half_t:tiles_per_chunk],
                )

    nc.compile()
    return nc


def prep_inputs(h, ret_feat, batch, Wp, bp, Wl, bl, shard_rows: int = SHARD,
                n_cores: int = N_CORES):
    """Host-side prep: shard + cast + pre-transpose h. Returns per-core maps."""
    h = np.asarray(h, dtype=np.float32)
    Wl = np.asarray(Wl, dtype=np.float32)
    Wp = np.asarray(Wp, dtype=np.float32)
    bp = np.asarray(bp, dtype=np.float32)
    bl = np.asarray(bl, dtype=np.float32)
    ret_feat = np.asarray(ret_feat, dtype=np.float32)

    h16 = h.astype(np.float16)
    bt_all = np.asarray(batch).astype(np.int64)

    wt16 = np.ascontiguousarray(Wl[:, :H].T).astype(np.float16)
    # replicated pooled ret table: ret2 = (mean_k rf) @ Wp.T + bp) @ Wl[:,H:].T + bl
    wlr_t = Wl[:, H:].astype(np.float64).T  # [R, H]
    ret = ret_feat.astype(np.float64).mean(axis=1) @ Wp.astype(np.float64).T + bp
    ret2 = ret @ wlr_t + bl  # [B, H] float64
    # stacked fp8 hi+lo table: rows 0-63 = fp8(ret2), 64-127 = fp8(residual)
    r2 = np.zeros((128, H), dtype=F8NP)
    r2[:B] = ret2.astype(F8NP)
    r2[B : 2 * B] = (ret2 - r2[:B].astype(np.float64)).astype(F8NP)

    # duplicated one-hot (rows b and 64+b set), fp8, streamed per core
    n_total = shard_rows * n_cores
    oh_all = np.zeros((128, n_total), dtype=F8NP)
    cols = np.arange(n_total)
    oh_all[bt_all, cols] = 1.0
    oh_all[bt_all + B, cols] = 1.0

    in_maps = []
    for i in range(n_cores):
        s = slice(i * shard_rows, (i + 1) * shard_rows)
        hs = h16[s]
        in_maps.append(
            {
                "hta": np.ascontiguousarray(hs[:, :128].T),
                "htb": np.ascontiguousarray(hs[:, 128:].T),
                "ohd": np.ascontiguousarray(oh_all[:, s]),
                "wt16": wt16,
                "r2": r2,
            }
        )
    return in_maps


_PROGRAM_CACHE = {}


def _get_program(shard_rows: int = SHARD):
    if shard_rows not in _PROGRAM_CACHE:
        _PROGRAM_CACHE[shard_rows] = build_program(shard_rows)
    return _PROGRAM_CACHE[shard_rows]


def kernel(h, ret_feat, batch, Wp, bp, Wl, bl):
    nc = _get_program(SHARD)
    in_maps = prep_inputs(h, ret_feat, batch, Wp, bp, Wl, bl)
    res = run_bass_kernel_spmd(nc, in_maps, list(range(N_CORES)))
    outs = []
    for i in range(N_CORES):
        ot = res.results[i]["out_t"]  # [128, n_tiles, H] fp16
        outs.append(ot.transpose(1, 0, 2).reshape(SHARD, H))
    return np.concatenate(outs, axis=0).astype(np.float32)


# revision 49
# speedup vs baseline: 1.0970x; 1.0194x over previous
"""Trainium2 Bass kernel for nn_Concatenation_90701119357422.

Computes, for full inputs:
    ret  = mean(ret_feat, axis=1) @ Wp.T + bp          # [B, H]
    out  = concat([h, ret[batch]], -1) @ Wl.T + bl     # [N, H]

Strategy (8 cores, data-parallel over N):
  - out = h @ Wl[:, :H].T + ret2[batch]  where  ret2 = ret @ Wl[:, H:].T + bl
  - host casts h to fp16 and pre-transposes it into two feature-major halves
    per core; device runs fp16 matmuls with fp32 PSUM accumulation
  - ret2 is computed on host (tiny) and replicated as a stacked fp8 hi+lo
    table [ret2hi; ret2lo] (error-compensated, near-fp16 precision)
  - per-row gather ret2[batch] is a single fp8 one-hot matmul accumulated
    into the same PSUM tile; the one-hot is built on HOST (rows 0-63 and
    64-127 both hold the one-hot, matching the stacked table) and streamed
    as fp8 -- no on-device broadcast/is_equal work
  - output is written fp16 in a feature-contiguous [128, tiles, H] layout;
    host de-transposes and upcasts to f32
"""

import os
import sys

import numpy as np

for _p in ("/opt/trn_rl_repo", "/root/.axon_site/_ro/trn_rl_repo"):
    if os.path.isdir(_p) and _p not in sys.path:
        sys.path.append(_p)

import concourse.bass as bass
import concourse.mybir as mybir
import concourse.tile as tile
from concourse import bacc
from concourse.bass_utils import run_bass_kernel_spmd

N_TOTAL = 262144
B = 64
K = 16
H = 256
R = 512
N_CORES = 8
SHARD = N_TOTAL // N_CORES  # 32768

CHUNK = 2048                 # rows per pipeline chunk
F32 = mybir.dt.float32
F16 = mybir.dt.float16
F8 = mybir.dt.float8e4
F8NP = mybir.dt.np(F8)


def build_program(shard_rows: int = SHARD):
    assert shard_rows % CHUNK == 0
    n_chunks = shard_rows // CHUNK
    tiles_per_chunk = CHUNK // 128
    n_tiles_total = shard_rows // 128

    nc = bacc.Bacc("TRN2", target_bir_lowering=False, debug=False)

    # feature-major fp16 h halves: hta[k, r] = h[r, k], htb[k, r] = h[r, 128+k]
    hta_d = nc.dram_tensor("hta", [128, shard_rows], F16, kind="ExternalInput").ap()
    htb_d = nc.dram_tensor("htb", [128, shard_rows], F16, kind="ExternalInput").ap()
    ohd = nc.dram_tensor("ohd", [64, shard_rows], F8, kind="ExternalInput").ap()
    wt16 = nc.dram_tensor("wt16", [H, H], F16, kind="ExternalInput").ap()
    r2_d = nc.dram_tensor("r2", [128, H], F16, kind="ExternalInput").ap()
    # out_t[p, t, n] = out[128*t + p, n], fp16; host de-transposes
    out_t = nc.dram_tensor(
        "out_t", [128, n_tiles_total, H], F16, kind="ExternalOutput"
    ).ap()

    with tile.TileContext(nc) as tc:
        with (
            tc.tile_pool(name="const", bufs=1) as cpool,
            tc.tile_pool(name="psum", bufs=1, space="PSUM") as ppool,
            tc.tile_pool(name="ht", bufs=6) as hpool,
            tc.tile_pool(name="oh", bufs=4) as ohpool,
            tc.tile_pool(name="outp", bufs=4) as opool,
        ):
            # ---- constants into SBUF ----
            wt_sb = cpool.tile([128, 2, H], F16)
            nc.scalar.dma_start(wt_sb[:], wt16.rearrange("(kc p) c -> p kc c", p=128))
            ret2_sb = cpool.tile([128, H], F16)
            nc.scalar.dma_start(ret2_sb[:], r2_d[:])

            # ---- main loop ----
            for ci in range(n_chunks):
                r0 = ci * CHUNK
                t0 = ci * tiles_per_chunk
                hta = hpool.tile([128, CHUNK], F16, tag="hta")
                nc.sync.dma_start(out=hta[:], in_=hta_d[:, r0 : r0 + CHUNK])
                htb = hpool.tile([128, CHUNK], F16, tag="htb")
                nc.sync.dma_start(out=htb[:], in_=htb_d[:, r0 : r0 + CHUNK])
                oh = ohpool.tile([128, CHUNK], F8, tag="oh")
                nc.sync.dma_start(out=oh[0:64, :], in_=ohd[:, r0 : r0 + CHUNK])
                # rows 64-127 zeroed on the idle Pool engine (SBUF-only op);
                # zero stationary rows nullify whatever ret2 rows they face
                nc.gpsimd.memset(oh[64:128, :], 0.0)

                outsb = opool.tile([128, tiles_per_chunk, H], F16, tag="outsb")
                half_t = tiles_per_chunk // 2
                for t in range(tiles_per_chunk):
                    ps = ppool.tile([128, H], F32, tag="acc", bufs=8)
                    sl = slice(128 * t, 128 * (t + 1))
                    nc.tensor.matmul(
                        ps[:], hta[:, sl], wt_sb[:, 0], start=True, stop=False
                    )
                    nc.tensor.matmul(
                        ps[:], htb[:, sl], wt_sb[:, 1], start=False, stop=False
                    )
                    nc.tensor.matmul(
                        ps[:], oh[:, sl], ret2_sb[:], start=False, stop=True
                    )
                    nc.any.tensor_copy(outsb[:, t], ps[:])
                    if t == half_t - 1:
                        # output triggers live on the (otherwise idle) gpsimd
                        # queue so their copy-waits never stall scalar's copies
                        nc.gpsimd.dma_start(
                            out=out_t[:, t0 : t0 + half_t, :],
                            in_=outsb[:, 0:half_t],
                        )

                nc.gpsimd.dma_start(
                    out=out_t[:, t0 + half_t : t0 + tiles_per_chunk, :],
                    in_=outsb[:, half_t:tiles_per_chunk],
                )

    nc.compile()
    return nc


def prep_inputs(h, ret_feat, batch, Wp, bp, Wl, bl, shard_rows: int = SHARD,
                n_cores: int = N_CORES):
    """Host-side prep: shard + cast + pre-transpose h. Returns per-core maps."""
    h = np.asarray(h, dtype=np.float32)
    Wl = np.asarray(Wl, dtype=np.float32)
    Wp = np.asarray(Wp, dtype=np.float32)
    bp = np.asarray(bp, dtype=np.float32)
    bl = np.asarray(bl, dtype=np.float32)
    ret_feat = np.asarray(ret_feat, dtype=np.float32)

    h16 = h.astype(np.float16)
    bt_all = np.asarray(batch).astype(np.int64)

    wt16 = np.ascontiguousarray(Wl[:, :H].T).astype(np.float16)
    # replicated pooled ret table: ret2 = (mean_k rf) @ Wp.T + bp) @ Wl[:,H:].T + bl
    wlr_t = Wl[:, H:].astype(np.float64).T  # [R, H]
    ret = ret_feat.astype(np.float64).mean(axis=1) @ Wp.astype(np.float64).T + bp
    ret2 = ret @ wlr_t + bl  # [B, H] float64
    # fp16 ret2 table (rows 64-127 zero, matching the zeroed one-hot rows)
    r2 = np.zeros((128, H), dtype=np.float16)
    r2[:B] = ret2.astype(np.float16)

    # single fp8 one-hot [64, N], streamed per core
    n_total = shard_rows * n_cores
    oh_all = np.zeros((64, n_total), dtype=F8NP)
    cols = np.arange(n_total)
    oh_all[bt_all, cols] = 1.0

    in_maps = []
    for i in range(n_cores):
        s = slice(i * shard_rows, (i + 1) * shard_rows)
        hs = h16[s]
        in_maps.append(
            {
                "hta": np.ascontiguousarray(hs[:, :128].T),
                "htb": np.ascontiguousarray(hs[:, 128:].T),
                "ohd": np.ascontiguousarray(oh_all[:, s]),
                "wt16": wt16,
                "r2": r2,
            }
        )
    return in_maps


_PROGRAM_CACHE = {}


def _get_program(shard_rows: int = SHARD):
    if shard_rows not in _PROGRAM_CACHE:
        _PROGRAM_CACHE[shard_rows] = build_program(shard_rows)
    return _PROGRAM_CACHE[shard_rows]


def kernel(h, ret_feat, batch, Wp, bp, Wl, bl):
    nc = _get_program(SHARD)
    in_maps = prep_inputs(h, ret_feat, batch, Wp, bp, Wl, bl)
    res = run_bass_kernel_spmd(nc, in_maps, list(range(N_CORES)))
    outs = []
    for i in range(N_CORES):
        ot = res.results[i]["out_t"]  # [128, n_tiles, H] fp16
        outs.append(ot.transpose(1, 0, 2).reshape(SHARD, H))
    return np.concatenate(outs, axis=0).astype(np.float32)
